# revision 3
# baseline (speedup 1.0000x reference)
"""Bidirectional-ALiBi bias kernel for Trainium2 (Bass/Tile), 8-core SPMD.

Computes out[h, i, j] = |j - i| * m where m = alpha[h] on the first
row/column, gamma[h] above the diagonal, beta[h] below it, and 0 on the
(non-edge) diagonal.  Output [16, 2048, 2048] f32, sharded 2 heads/core.

Strategy: every interior row i is a shifted window of a per-head profile
V(k) = gamma*max(k,0) + beta*max(-k,0), k = j - i.  Each core
materializes diagonalized SBUF chunk tiles Wg[p, c-lo] = V(c - p - 2047)
(chunks overlap by EXT columns so every output DMA piece can be served
from a single tile); row-block t of the output reads the window
c in [2176-128t, 4095-128t) for j in [129, 2048).

Column edge (j < 129): a per-head "strip mega" tile SM holds, per block,
129 columns = [alpha*i | V values for j=1..128]; strips are written as
separate [128, 129] DMAs (516 B rows — every DMA descriptor in this
kernel is >= 512 B, because sub-512 B descriptors make the SDMA engines
do a read-modify-write round trip per descriptor to HBM, which was
measured to crater write throughput from ~400 to ~180 GB/s).
Row 0 (alpha edge) is patched into SM (block 0 slot) and into a
dedicated W2 tile that covers block 0's j in [129, 2048).

Engine placement: gpsimd runs the K iotas and the first half of the
V computation (T2 = max(gamma*K, 0)); DVE runs the second half
(W = max(-beta*K, T2)) plus patch/copy ops.  The Activation engine is
left free on purpose: nc.scalar is one of the two HWDGE DMA-trigger
rings, and compute there would stall ring-2 DMA triggers (rings are
FIFO).  A small first chunk (256 cols) gets the first DMAs going ~1.5us
after the start barrier instead of ~10us.

Hardware notes (from NTFF profiling): 16 SDMA engines, ~26.5 GB/s each;
HBM-per-core limit ~358 GB/s; the 33.6 MB/core of output writes set a
~94 us roofline.  DMAs alternate between the SP and Activation HWDGE
rings; emission order per ring matches data-availability order since
rings are head-of-line blocking.
"""

import numpy as np

H = 16
S = 2048
P = 128
N_CORES = 8
H_LOC = H // N_CORES  # 2 heads per core
WID = 2 * S - 1  # profile width; c in [0, WID), k = c - p - (S-1)
NT = S // P  # 16 row blocks per head

EXT = 127  # chunk overlap: lets pieces cross a base boundary by < 128 cols
SW = 129  # strip width (col 0 + 128 interior cols); 516 B rows

# chunk base bounds in compute order: mini, c2rest, c1, c3, c0
CHUNKS = [(2048, 2304), (2304, 3072), (1024, 2048), (3072, 4095), (0, 1024)]
MEGA_AFTER = 1  # compute SM (strips) after this chunk index
STRIP_FROM = 2  # interleave strip DMAs into phases >= this index

_NC = None


def _pieces_for_block(t):
    """Interior DMA pieces for row block t: list of (a, b, chunk_idx)
    covering c in [2176-128t, 4095-128t); each piece is served by one
    (extended) chunk tile and is always >= 128 cols (512 B rows)."""
    ws, we = 2176 - 128 * t, 4095 - 128 * t
    cuts = sorted({lo for lo, _ in CHUNKS} | {hi for _, hi in CHUNKS})
    bounds = [ws] + [c for c in cuts if ws < c < we] + [we]
    pieces = []
    for a, b in zip(bounds[:-1], bounds[1:]):
        if b - a < P and pieces:  # merge short tail into previous piece
            a = pieces.pop()[0]
        gi = next(i for i, (lo, hi) in enumerate(CHUNKS) if lo <= a < hi)
        assert b <= min(CHUNKS[gi][1] + EXT, WID)
        pieces.append((a, b, gi))
    return pieces


def _build(t2_on_gpsimd=True, sw=SW, mega_after=MEGA_AFTER, strip_from=STRIP_FROM):
    import concourse.bacc as bacc
    import concourse.mybir as mybir
    from concourse.tile import TileContext

    f32 = mybir.dt.float32
    nc = bacc.Bacc("TRN2", target_bir_lowering=False, debug=False)

    alpha_d = nc.dram_tensor("alpha", [H_LOC], f32, kind="ExternalInput").ap()
    beta_d = nc.dram_tensor("beta", [H_LOC], f32, kind="ExternalInput").ap()
    gamma_d = nc.dram_tensor("gamma", [H_LOC], f32, kind="ExternalInput").ap()
    out_d = nc.dram_tensor("out", [H_LOC, S, S], f32, kind="ExternalOutput").ap()

    KW = max(min(hi + EXT, WID) - lo for lo, hi in CHUNKS)  # widest chunk tile
    MW = NT * sw  # strip mega width

    # phase_pieces[gi][h] = [(t, a, b), ...]
    phase_pieces = [[[], []] for _ in CHUNKS]
    for t in range(1, NT):
        for a, b, gi in _pieces_for_block(t):
            for h in range(H_LOC):
                phase_pieces[gi][h].append((t, a, b))

    ring_i = 0

    with TileContext(nc) as tc:
        rings = [nc.sync, nc.scalar]

        def dma(out, in_):
            nonlocal ring_i
            rings[ring_i % 2].dma_start(out=out, in_=in_)
            ring_i += 1

        with (
            tc.tile_pool(name="coef", bufs=1) as cpool,
            tc.tile_pool(name="kpool", bufs=len(CHUNKS)) as kpool,
            tc.tile_pool(name="kmega", bufs=1) as kmpool,
            tc.tile_pool(name="tpool", bufs=2) as tpool,
            tc.tile_pool(name="tmega", bufs=1) as tmpool,
            tc.tile_pool(name="wpool", bufs=len(CHUNKS)) as wpool,
            tc.tile_pool(name="w2pool", bufs=1) as w2pool,
            tc.tile_pool(name="smpool", bufs=1) as smpool,
        ):
            t2eng = nc.gpsimd if t2_on_gpsimd else nc.vector

            # per-head coefficients broadcast to all partitions: [128, 2].
            # These land on the runtime queue during the NEFF start barrier,
            # so they are ready before the first compute op.
            G2 = cpool.tile([P, H_LOC], f32)
            nc.sync.dma_start(out=G2[:], in_=gamma_d.partition_broadcast(P))
            B2 = cpool.tile([P, H_LOC], f32)
            nc.scalar.dma_start(out=B2[:], in_=beta_d.partition_broadcast(P))
            A2 = cpool.tile([P, H_LOC], f32)
            nc.sync.dma_start(out=A2[:], in_=alpha_d.partition_broadcast(P))
            NB2 = cpool.tile([P, H_LOC], f32)
            nc.vector.tensor_scalar_mul(NB2[:], B2[:], -1.0)

            Ks = {}  # gi -> (tile, lo, width)
            Ws = {}  # (h, gi) -> W tile
            strips_emitted = 0
            sm_tiles = {}

            def emit_strip_dmas(n):
                nonlocal strips_emitted
                while n > 0 and strips_emitted < H_LOC * NT:
                    h, s = divmod(strips_emitted, NT)
                    t = NT - 1 - s
                    dma(
                        out=out_d[h, P * t : P * (t + 1), 0:sw],
                        in_=sm_tiles[h][:, sw * s : sw * (s + 1)],
                    )
                    strips_emitted += 1
                    n -= 1

            def build_mega():
                # SM[p, sw*s + u]: u=0 -> alpha*(128*(15-s) + p);
                # u>=1 -> V(u + 128*s - 1920 - p)  (block t = 15-s, j = u)
                IBrev = cpool.tile([P, NT], f32, tag="IBrev")
                nc.gpsimd.iota(
                    IBrev[:],
                    pattern=[[-P, NT]],
                    base=(NT - 1) * P,
                    channel_multiplier=1,
                    allow_small_or_imprecise_dtypes=True,
                )
                Km = kmpool.tile([P, MW], f32, tag="Kmega")
                nc.gpsimd.iota(
                    Km[:],
                    pattern=[[P, NT], [1, sw]],
                    base=-(NT - 1) * P,
                    channel_multiplier=-1,
                    allow_small_or_imprecise_dtypes=True,
                )
                for h in range(H_LOC):
                    T2m = tmpool.tile([P, MW], f32, tag=f"T2m{h}")
                    t2eng.tensor_scalar(
                        out=T2m[:],
                        in0=Km[:],
                        scalar1=G2[:, h : h + 1],
                        scalar2=0.0,
                        op0=mybir.AluOpType.mult,
                        op1=mybir.AluOpType.max,
                    )
                    SM = smpool.tile([P, MW], f32, tag=f"SM{h}", name=f"SM{h}")
                    nc.vector.scalar_tensor_tensor(
                        out=SM[:],
                        in0=Km[:],
                        scalar=NB2[:, h : h + 1],
                        in1=T2m[:],
                        op0=mybir.AluOpType.mult,
                        op1=mybir.AluOpType.max,
                    )
                    # col-0 slots: SM[p, sw*s] = alpha_h * IBrev[p, s]
                    smv = SM[:].rearrange("p (s u) -> p s u", u=sw)
                    nc.vector.tensor_scalar_mul(
                        smv[:, :, 0:1], IBrev[:].unsqueeze(2), A2[:, h : h + 1]
                    )
                    # block-0 (slot s=15) row 0, j>=1: alpha_h * j; Km row
                    # p=0 in that slot holds exactly j.
                    nc.vector.tensor_scalar_mul(
                        SM[0:1, sw * (NT - 1) + 1 : MW],
                        Km[0:1, sw * (NT - 1) + 1 : MW],
                        A2[0:1, h : h + 1],
                    )
                    sm_tiles[h] = SM

            def build_w2(h):
                # block 0, j in [129, 2048) -> c in [2176, 4095); row 0 is
                # alpha*j with j = c - 2047 (K rows hold exactly that).
                W2 = w2pool.tile([P, S - sw], f32, tag=f"W2{h}")
                c0 = S + sw - 1  # 2176
                for src_gi, lo_c, hi_c in ((0, c0, 2304), (1, 2304, 3072), (3, 3072, WID)):
                    Kg, lo, _ = Ks[src_gi]
                    d0, wC = lo_c - c0, hi_c - lo_c
                    nc.vector.tensor_copy(
                        out=W2[:, d0 : d0 + wC],
                        in_=Ws[(h, src_gi)][:, lo_c - lo : hi_c - lo],
                    )
                    nc.vector.tensor_scalar_mul(
                        W2[0:1, d0 : d0 + wC],
                        Kg[0:1, lo_c - lo : hi_c - lo],
                        A2[0:1, h : h + 1],
                    )
                return W2

            for gi, (lo, hi) in enumerate(CHUNKS):
                hi_e = min(hi + EXT, WID)
                w = hi_e - lo
                Kg = kpool.tile([P, KW], f32, tag="K")
                nc.gpsimd.iota(
                    Kg[:, :w],
                    pattern=[[1, w]],
                    base=lo - (S - 1),
                    channel_multiplier=-1,
                    allow_small_or_imprecise_dtypes=True,
                )
                Ks[gi] = (Kg, lo, w)
                for h in range(H_LOC):
                    # T2 = max(gamma*k, 0); W = max(-beta*k, T2) == V(k)
                    T2 = tpool.tile([P, KW], f32, tag=f"T2{h}")
                    t2eng.tensor_scalar(
                        out=T2[:, :w],
                        in0=Kg[:, :w],
                        scalar1=G2[:, h : h + 1],
                        scalar2=0.0,
                        op0=mybir.AluOpType.mult,
                        op1=mybir.AluOpType.max,
                    )
                    Wt = wpool.tile([P, KW], f32, tag=f"W{h}")
                    nc.vector.scalar_tensor_tensor(
                        out=Wt[:, :w],
                        in0=Kg[:, :w],
                        scalar=NB2[:, h : h + 1],
                        in1=T2[:, :w],
                        op0=mybir.AluOpType.mult,
                        op1=mybir.AluOpType.max,
                    )
                    Ws[(h, gi)] = Wt

                    # this chunk's interior piece DMAs for head h
                    for t, a, b in phase_pieces[gi][h]:
                        j_lo = a - (S - 1) + P * t
                        j_hi = b - (S - 1) + P * t
                        dma(
                            out=out_d[h, P * t : P * (t + 1), j_lo:j_hi],
                            in_=Wt[:, a - lo : b - lo],
                        )
                        if gi >= strip_from:
                            emit_strip_dmas(1)

                if gi == mega_after:
                    build_mega()
                if gi == 3:  # c3 done -> W2 buildable
                    for h in range(H_LOC):
                        W2 = build_w2(h)
                        dma(out=out_d[h, 0:P, sw:S], in_=W2[:])
                        emit_strip_dmas(1)

            emit_strip_dmas(H_LOC * NT)  # any leftovers

    nc.compile()
    return nc


def _run(alpha, beta, gamma, **spmd_kwargs):
    """Compile (cached) and run on the 8 NeuronCores; returns BassKernelResults."""
    global _NC
    if _NC is None:
        _NC = _build()
    from concourse import bass_utils

    alpha = np.ascontiguousarray(alpha, dtype=np.float32)
    beta = np.ascontiguousarray(beta, dtype=np.float32)
    gamma = np.ascontiguousarray(gamma, dtype=np.float32)
    in_maps = [
        {
            "alpha": alpha[c * H_LOC : (c + 1) * H_LOC],
            "beta": beta[c * H_LOC : (c + 1) * H_LOC],
            "gamma": gamma[c * H_LOC : (c + 1) * H_LOC],
        }
        for c in range(N_CORES)
    ]
    return bass_utils.run_bass_kernel_spmd(
        _NC, in_maps, core_ids=list(range(N_CORES)), **spmd_kwargs
    )


def kernel(alpha, beta, gamma, seq_len):
    assert int(seq_len) == S, f"kernel hardcodes seq_len={S}, got {seq_len}"
    res = _run(alpha, beta, gamma)
    return np.concatenate([r["out"] for r in res.results], axis=0)


# revision 5
# speedup vs baseline: 1.9476x; 1.9476x over previous
"""Bidirectional-ALiBi bias kernel for Trainium2 (Bass/Tile), 8-core SPMD.

Computes out[h, i, j] = |j - i| * m where m = alpha[h] on the first
row/column, gamma[h] above the diagonal, beta[h] below it, and 0 on the
(non-edge) diagonal.  Output [16, 2048, 2048] f32, sharded 2 heads/core.

Strategy: every interior row i is a shifted window of a per-head profile
V(k) = gamma*max(k,0) + beta*max(-k,0), k = j - i.  Each core
materializes diagonalized SBUF chunk tiles Wg[p, c-lo] = V(c - p - 2047)
(chunks overlap by EXT columns so every output DMA piece can be served
from a single tile); row-block t of the output reads the window
c in [2176-128t, 4095-128t) for j in [129, 2048).

Column edge (j < 129): a per-head "strip mega" tile SM holds, per block,
129 columns = [alpha*i | V values for j=1..128]; strips are written as
separate [128, 129] DMAs (516 B rows — every DMA descriptor in this
kernel is >= 512 B, because sub-512 B descriptors make the SDMA engines
do a read-modify-write round trip per descriptor to HBM, which was
measured to crater write throughput from ~400 to ~180 GB/s).
Row 0 (alpha edge) is patched into SM (block 0 slot) and into a
dedicated W2 tile that covers block 0's j in [129, 2048).

Engine placement: gpsimd runs ONLY the K iotas; all elementwise compute
(T2 = max(gamma*K, 0), W = max(-beta*K, T2), copies, patches) runs on
DVE.  GpSimd and a DVE 2-port perf-mode op (tensor_scalar/copy) FULLY
BLOCK each other on the shared SBUF port pair — putting T2 on gpsimd
to "parallelize" was measured to 2x the total runtime.  The Activation
engine is left free on purpose: nc.scalar is one of the two HWDGE
DMA-trigger rings, and compute there would stall ring-2 DMA triggers
(rings are FIFO).  A small first chunk (256 cols) gets the first DMAs
going early instead of ~10us in.

Hardware notes (from NTFF profiling): 16 SDMA engines, ~26.5 GB/s each;
HBM-per-core limit ~358 GB/s; the 33.6 MB/core of output writes set a
~94 us roofline.  DMAs alternate between the SP and Activation HWDGE
rings; emission order per ring matches data-availability order since
rings are head-of-line blocking.
"""

import numpy as np

H = 16
S = 2048
P = 128
N_CORES = 8
H_LOC = H // N_CORES  # 2 heads per core
WID = 2 * S - 1  # profile width; c in [0, WID), k = c - p - (S-1)
NT = S // P  # 16 row blocks per head

EXT = 127  # chunk overlap: lets pieces cross a base boundary by < 128 cols
SW = 129  # strip width (col 0 + 128 interior cols); 516 B rows

# chunk base bounds in compute order: mini, c2rest, c1, c3, c0
CHUNKS = [(2048, 2304), (2304, 3072), (1024, 2048), (3072, 4095), (0, 1024)]
MEGA_AFTER = 1  # compute SM (strips) after this chunk index
STRIP_FROM = 2  # interleave strip DMAs into phases >= this index

_NC = None


def _pieces_for_block(t):
    """Interior DMA pieces for row block t: list of (a, b, chunk_idx)
    covering c in [2176-128t, 4095-128t); each piece is served by one
    (extended) chunk tile and is always >= 128 cols (512 B rows)."""
    ws, we = 2176 - 128 * t, 4095 - 128 * t
    cuts = sorted({lo for lo, _ in CHUNKS} | {hi for _, hi in CHUNKS})
    bounds = [ws] + [c for c in cuts if ws < c < we] + [we]
    pieces = []
    for a, b in zip(bounds[:-1], bounds[1:]):
        if b - a < P and pieces:  # merge short tail into previous piece
            a = pieces.pop()[0]
        gi = next(i for i, (lo, hi) in enumerate(CHUNKS) if lo <= a < hi)
        assert b <= min(CHUNKS[gi][1] + EXT, WID)
        pieces.append((a, b, gi))
    return pieces


def _build(t2_on_gpsimd=False, sw=SW, mega_after=MEGA_AFTER, strip_from=STRIP_FROM):
    import concourse.bacc as bacc
    import concourse.mybir as mybir
    from concourse.tile import TileContext

    f32 = mybir.dt.float32
    nc = bacc.Bacc("TRN2", target_bir_lowering=False, debug=False)

    alpha_d = nc.dram_tensor("alpha", [H_LOC], f32, kind="ExternalInput").ap()
    beta_d = nc.dram_tensor("beta", [H_LOC], f32, kind="ExternalInput").ap()
    gamma_d = nc.dram_tensor("gamma", [H_LOC], f32, kind="ExternalInput").ap()
    out_d = nc.dram_tensor("out", [H_LOC, S, S], f32, kind="ExternalOutput").ap()

    KW = max(min(hi + EXT, WID) - lo for lo, hi in CHUNKS)  # widest chunk tile
    MW = NT * sw  # strip mega width

    # phase_pieces[gi][h] = [(t, a, b), ...]
    phase_pieces = [[[], []] for _ in CHUNKS]
    for t in range(1, NT):
        for a, b, gi in _pieces_for_block(t):
            for h in range(H_LOC):
                phase_pieces[gi][h].append((t, a, b))

    ring_i = 0

    with TileContext(nc) as tc:
        rings = [nc.sync, nc.scalar]

        def dma(out, in_):
            nonlocal ring_i
            rings[ring_i % 2].dma_start(out=out, in_=in_)
            ring_i += 1

        with (
            tc.tile_pool(name="coef", bufs=1) as cpool,
            tc.tile_pool(name="kpool", bufs=len(CHUNKS)) as kpool,
            tc.tile_pool(name="kmega", bufs=1) as kmpool,
            tc.tile_pool(name="tpool", bufs=2) as tpool,
            tc.tile_pool(name="tmega", bufs=1) as tmpool,
            tc.tile_pool(name="wpool", bufs=len(CHUNKS)) as wpool,
            tc.tile_pool(name="w2pool", bufs=1) as w2pool,
            tc.tile_pool(name="smpool", bufs=1) as smpool,
        ):
            t2eng = nc.gpsimd if t2_on_gpsimd else nc.vector

            # per-head coefficients broadcast to all partitions: [128, 2].
            # These land on the runtime queue during the NEFF start barrier,
            # so they are ready before the first compute op.
            G2 = cpool.tile([P, H_LOC], f32)
            nc.sync.dma_start(out=G2[:], in_=gamma_d.partition_broadcast(P))
            B2 = cpool.tile([P, H_LOC], f32)
            nc.scalar.dma_start(out=B2[:], in_=beta_d.partition_broadcast(P))
            A2 = cpool.tile([P, H_LOC], f32)
            nc.sync.dma_start(out=A2[:], in_=alpha_d.partition_broadcast(P))
            NB2 = cpool.tile([P, H_LOC], f32)
            nc.vector.tensor_scalar_mul(NB2[:], B2[:], -1.0)

            Ks = {}  # gi -> (tile, lo, width)
            Ws = {}  # (h, gi) -> W tile
            strips_emitted = 0
            sm_tiles = {}

            def emit_strip_dmas(n):
                nonlocal strips_emitted
                while n > 0 and strips_emitted < H_LOC * NT:
                    h, s = divmod(strips_emitted, NT)
                    t = NT - 1 - s
                    dma(
                        out=out_d[h, P * t : P * (t + 1), 0:sw],
                        in_=sm_tiles[h][:, sw * s : sw * (s + 1)],
                    )
                    strips_emitted += 1
                    n -= 1

            def build_mega():
                # SM[p, sw*s + u]: u=0 -> alpha*(128*(15-s) + p);
                # u>=1 -> V(u + 128*s - 1920 - p)  (block t = 15-s, j = u)
                IBrev = cpool.tile([P, NT], f32, tag="IBrev")
                nc.gpsimd.iota(
                    IBrev[:],
                    pattern=[[-P, NT]],
                    base=(NT - 1) * P,
                    channel_multiplier=1,
                    allow_small_or_imprecise_dtypes=True,
                )
                Km = kmpool.tile([P, MW], f32, tag="Kmega")
                nc.gpsimd.iota(
                    Km[:],
                    pattern=[[P, NT], [1, sw]],
                    base=-(NT - 1) * P,
                    channel_multiplier=-1,
                    allow_small_or_imprecise_dtypes=True,
                )
                for h in range(H_LOC):
                    T2m = tmpool.tile([P, MW], f32, tag=f"T2m{h}")
                    t2eng.tensor_scalar(
                        out=T2m[:],
                        in0=Km[:],
                        scalar1=G2[:, h : h + 1],
                        scalar2=0.0,
                        op0=mybir.AluOpType.mult,
                        op1=mybir.AluOpType.max,
                    )
                    SM = smpool.tile([P, MW], f32, tag=f"SM{h}", name=f"SM{h}")
                    nc.vector.scalar_tensor_tensor(
                        out=SM[:],
                        in0=Km[:],
                        scalar=NB2[:, h : h + 1],
                        in1=T2m[:],
                        op0=mybir.AluOpType.mult,
                        op1=mybir.AluOpType.max,
                    )
                    # col-0 slots: SM[p, sw*s] = alpha_h * IBrev[p, s]
                    smv = SM[:].rearrange("p (s u) -> p s u", u=sw)
                    nc.vector.tensor_scalar_mul(
                        smv[:, :, 0:1], IBrev[:].unsqueeze(2), A2[:, h : h + 1]
                    )
                    # block-0 (slot s=15) row 0, j>=1: alpha_h * j; Km row
                    # p=0 in that slot holds exactly j.
                    nc.vector.tensor_scalar_mul(
                        SM[0:1, sw * (NT - 1) + 1 : MW],
                        Km[0:1, sw * (NT - 1) + 1 : MW],
                        A2[0:1, h : h + 1],
                    )
                    sm_tiles[h] = SM

            def build_w2(h):
                # block 0, j in [129, 2048) -> c in [2176, 4095); row 0 is
                # alpha*j with j = c - 2047 (K rows hold exactly that).
                W2 = w2pool.tile([P, S - sw], f32, tag=f"W2{h}")
                c0 = S + sw - 1  # 2176
                for src_gi, lo_c, hi_c in ((0, c0, 2304), (1, 2304, 3072), (3, 3072, WID)):
                    Kg, lo, _ = Ks[src_gi]
                    d0, wC = lo_c - c0, hi_c - lo_c
                    nc.vector.tensor_copy(
                        out=W2[:, d0 : d0 + wC],
                        in_=Ws[(h, src_gi)][:, lo_c - lo : hi_c - lo],
                    )
                    nc.vector.tensor_scalar_mul(
                        W2[0:1, d0 : d0 + wC],
                        Kg[0:1, lo_c - lo : hi_c - lo],
                        A2[0:1, h : h + 1],
                    )
                return W2

            for gi, (lo, hi) in enumerate(CHUNKS):
                hi_e = min(hi + EXT, WID)
                w = hi_e - lo
                Kg = kpool.tile([P, KW], f32, tag="K")
                nc.gpsimd.iota(
                    Kg[:, :w],
                    pattern=[[1, w]],
                    base=lo - (S - 1),
                    channel_multiplier=-1,
                    allow_small_or_imprecise_dtypes=True,
                )
                Ks[gi] = (Kg, lo, w)
                for h in range(H_LOC):
                    # T2 = max(gamma*k, 0); W = max(-beta*k, T2) == V(k)
                    T2 = tpool.tile([P, KW], f32, tag=f"T2{h}")
                    t2eng.tensor_scalar(
                        out=T2[:, :w],
                        in0=Kg[:, :w],
                        scalar1=G2[:, h : h + 1],
                        scalar2=0.0,
                        op0=mybir.AluOpType.mult,
                        op1=mybir.AluOpType.max,
                    )
                    Wt = wpool.tile([P, KW], f32, tag=f"W{h}")
                    nc.vector.scalar_tensor_tensor(
                        out=Wt[:, :w],
                        in0=Kg[:, :w],
                        scalar=NB2[:, h : h + 1],
                        in1=T2[:, :w],
                        op0=mybir.AluOpType.mult,
                        op1=mybir.AluOpType.max,
                    )
                    Ws[(h, gi)] = Wt

                    # this chunk's interior piece DMAs for head h
                    for t, a, b in phase_pieces[gi][h]:
                        j_lo = a - (S - 1) + P * t
                        j_hi = b - (S - 1) + P * t
                        dma(
                            out=out_d[h, P * t : P * (t + 1), j_lo:j_hi],
                            in_=Wt[:, a - lo : b - lo],
                        )
                        if gi >= strip_from:
                            emit_strip_dmas(1)

                if gi == mega_after:
                    build_mega()
                if gi == 3:  # c3 done -> W2 buildable
                    for h in range(H_LOC):
                        W2 = build_w2(h)
                        dma(out=out_d[h, 0:P, sw:S], in_=W2[:])
                        emit_strip_dmas(1)

            emit_strip_dmas(H_LOC * NT)  # any leftovers

    nc.compile()
    return nc


def _run(alpha, beta, gamma, **spmd_kwargs):
    """Compile (cached) and run on the 8 NeuronCores; returns BassKernelResults."""
    global _NC
    if _NC is None:
        _NC = _build()
    from concourse import bass_utils

    alpha = np.ascontiguousarray(alpha, dtype=np.float32)
    beta = np.ascontiguousarray(beta, dtype=np.float32)
    gamma = np.ascontiguousarray(gamma, dtype=np.float32)
    in_maps = [
        {
            "alpha": alpha[c * H_LOC : (c + 1) * H_LOC],
            "beta": beta[c * H_LOC : (c + 1) * H_LOC],
            "gamma": gamma[c * H_LOC : (c + 1) * H_LOC],
        }
        for c in range(N_CORES)
    ]
    return bass_utils.run_bass_kernel_spmd(
        _NC, in_maps, core_ids=list(range(N_CORES)), **spmd_kwargs
    )


def kernel(alpha, beta, gamma, seq_len):
    assert int(seq_len) == S, f"kernel hardcodes seq_len={S}, got {seq_len}"
    res = _run(alpha, beta, gamma)
    return np.concatenate([r["out"] for r in res.results], axis=0)


# revision 11
# speedup vs baseline: 2.1182x; 1.0876x over previous
"""Bidirectional-ALiBi bias kernel for Trainium2 (Bass/Tile), 8-core SPMD.

Computes out[h, i, j] = |j - i| * m where m = alpha[h] on the first
row/column, gamma[h] above the diagonal, beta[h] below it, and 0 on the
(non-edge) diagonal.  Output [16, 2048, 2048] f32, sharded 2 heads/core.

Strategy: every interior row i is a shifted window of the per-head
profile V(k) = gamma*max(k,0) + beta*max(-k,0), k = j - i.  Each core
computes diagonalized chunk tiles Wg[p, c-lo] = V(c - p - 2047) (four
1024-col chunks), then assembles, per 128-row block t, two contiguous
half-row tiles  QL = [alpha*i | V cols j=1..1023]  and
QR = [V cols j=1024..2047]  and streams them out as plain [128, 1024]
DMAs (4 KB rows).  Block 0's row 0 is patched to alpha*j in-place.

Why assemble-and-copy instead of DMAing straight out of the W chunks:
HBM write efficiency at 8 KB row stride is set by descriptor size —
4-8 KB descriptors sustain ~400 GB/s, while sub-1 KB descriptors pay a
DRAM row-activation (and <512 B an SDMA read-modify-write) per row and
were measured to collapse throughput to 150-250 GB/s.  Copying through
SBUF makes every single descriptor in the kernel a 4 KB row.

Engine placement (respects the DVE/GpSimd shared-SBUF-port lock and
keeps both HWDGE rings' trigger queues self-paced):
  gpsimd: coefficient partition_broadcast, IB iota, ONE master K iota
  ACT (nc.scalar): derives the other K chunks (K +- const), copies all
      RIGHT halves, and triggers their DMAs on the ACT HWDGE ring
  DVE: T2/W chunk compute, all LEFT halves + patches
  SP ring (nc.sync): triggers LEFT-half DMAs
Left and right halves are 16.8 MB each - the two rings stay balanced.

Hardware notes (from NTFF profiling): 16 SDMA engines, ~26.5 GB/s each;
HBM-per-core limit ~358 GB/s; 33.6 MB/core of output writes set a
~94 us roofline (plus ~6.5 us fixed NEFF start barrier and ~10 us of
pipeline fill/drain).
"""

import numpy as np

H = 16
S = 2048
P = 128
N_CORES = 8
H_LOC = H // N_CORES  # 2 heads per core
NT = S // P  # 16 row blocks per head
HW = 1024  # half-row width

# chunk bounds in compute order: c2 (master iota), c1, c3, c0
CHQ = [(2048, 3072), (1024, 2048), (3072, 4095), (0, 1024)]

_NC = None


def _half_pieces(t, right):
    """Pieces (a, b, gi) of c-range for block t's half; j = c - 2047 + 128t.
    Left half covers j in [1, 1024), right half j in [1024, 2048)."""
    if right:
        ws, we = 3071 - 128 * t, 4095 - 128 * t
    else:
        ws, we = 2048 - 128 * t, 3071 - 128 * t
    cuts = sorted({b for lo, hi in CHQ for b in (lo, hi)})
    bounds = [ws] + [c for c in cuts if ws < c < we] + [we]
    out = []
    for a, b in zip(bounds[:-1], bounds[1:]):
        gi = next(i for i, (lo, hi) in enumerate(CHQ) if lo <= a < hi)
        assert b <= CHQ[gi][1]
        out.append((a, b, gi))
    return out


def _build():
    import concourse.bacc as bacc
    import concourse.mybir as mybir
    from concourse.tile import TileContext

    f32 = mybir.dt.float32
    Copy = mybir.ActivationFunctionType.Copy
    Ident = mybir.ActivationFunctionType.Identity
    nc = bacc.Bacc("TRN2", target_bir_lowering=False, debug=False)

    alpha_d = nc.dram_tensor("alpha", [H_LOC], f32, kind="ExternalInput").ap()
    beta_d = nc.dram_tensor("beta", [H_LOC], f32, kind="ExternalInput").ap()
    gamma_d = nc.dram_tensor("gamma", [H_LOC], f32, kind="ExternalInput").ap()
    out_d = nc.dram_tensor("out", [H_LOC, S, S], f32, kind="ExternalOutput").ap()

    with TileContext(nc) as tc:
        with (
            tc.tile_pool(name="coef", bufs=1) as cpool,
            tc.tile_pool(name="kpool", bufs=len(CHQ)) as kpool,
            tc.tile_pool(name="tpool", bufs=1) as tpool,
            tc.tile_pool(name="wpool", bufs=len(CHQ)) as wpool,
            tc.tile_pool(name="qlpool", bufs=5) as qlpool,
            tc.tile_pool(name="qrpool", bufs=5) as qrpool,
        ):
            # --- coefficients: one 24 B DMA row + on-chip broadcast ---
            # layout: C1[0, 0:2]=alpha, [2:4]=beta, [4:6]=gamma (per head)
            C1 = cpool.tile([1, 3 * H_LOC], f32, tag="C1")
            nc.sync.dma_start(out=C1[0:1, 0:H_LOC], in_=alpha_d.unsqueeze(0))
            nc.sync.dma_start(out=C1[0:1, H_LOC : 2 * H_LOC], in_=beta_d.unsqueeze(0))
            nc.sync.dma_start(out=C1[0:1, 2 * H_LOC :], in_=gamma_d.unsqueeze(0))
            CB = cpool.tile([P, 3 * H_LOC], f32, tag="CB")
            nc.gpsimd.partition_broadcast(CB[:], C1[0:1, :])
            A2, B2, G2 = CB[:, 0:H_LOC], CB[:, H_LOC : 2 * H_LOC], CB[:, 2 * H_LOC :]
            NB2 = cpool.tile([P, H_LOC], f32, tag="NB2")
            nc.vector.tensor_scalar_mul(NB2[:], B2, -1.0)

            # IB[p, t] = 128t + p ; Rs_h[p, t] = alpha_h * (128t + p)
            IB = cpool.tile([P, NT], f32, tag="IB")
            nc.gpsimd.iota(
                IB[:],
                pattern=[[P, NT]],
                base=0,
                channel_multiplier=1,
                allow_small_or_imprecise_dtypes=True,
            )
            Rs = {}
            for h in range(H_LOC):
                Rh = cpool.tile([P, NT], f32, tag=f"Rs{h}")
                nc.vector.tensor_scalar_mul(Rh[:], IB[:], A2[:, h : h + 1])
                Rs[h] = Rh

            # --- K chunks: one master iota, others derived on ACT ---
            Ks = {}
            lo0 = CHQ[0][0]
            K0 = kpool.tile([P, HW], f32, tag="K")
            nc.gpsimd.iota(
                K0[:],
                pattern=[[1, HW]],
                base=lo0 - (S - 1),
                channel_multiplier=-1,
                allow_small_or_imprecise_dtypes=True,
            )
            Ks[0] = K0
            for gi, (lo, hi) in list(enumerate(CHQ))[1:]:
                Kg = kpool.tile([P, HW], f32, tag="K")
                w = hi - lo
                bias = cpool.tile([P, 1], f32, tag=f"bias{gi}")
                nc.gpsimd.memset(bias[:], float(lo - lo0))
                nc.scalar.activation(
                    out=Kg[:, :w], in_=K0[:, :w], func=Ident, bias=bias[:]
                )
                Ks[gi] = Kg

            # --- W chunks: T2 = max(gamma*k, 0); W = max(-beta*k, T2) ---
            Ws = {}

            def compute_chunk(gi):
                lo, hi = CHQ[gi]
                w = hi - lo
                for h in range(H_LOC):
                    T2 = tpool.tile([P, HW], f32, tag=f"T2{h}")
                    nc.vector.tensor_scalar(
                        out=T2[:, :w],
                        in0=Ks[gi][:, :w],
                        scalar1=G2[:, h : h + 1],
                        scalar2=0.0,
                        op0=mybir.AluOpType.mult,
                        op1=mybir.AluOpType.max,
                    )
                    Wt = wpool.tile([P, HW], f32, tag=f"W{h}")
                    nc.vector.scalar_tensor_tensor(
                        out=Wt[:, :w],
                        in0=Ks[gi][:, :w],
                        scalar=NB2[:, h : h + 1],
                        in1=T2[:, :w],
                        op0=mybir.AluOpType.mult,
                        op1=mybir.AluOpType.max,
                    )
                    Ws[(h, gi)] = Wt

            def emit_half(h, t, right):
                """Copy pieces into a fresh contiguous half tile, patch
                edges, DMA it out.  Left: DVE copies + SP-ring trigger;
                right: ACT copies + ACT-ring trigger."""
                ring = nc.scalar if right else nc.sync
                pool = qrpool if right else qlpool
                Q = pool.tile([P, HW], f32, tag=f"Q{'R' if right else 'L'}{h}")
                j0 = HW if right else 0  # output column of Q col 0
                for a, b, gi in _half_pieces(t, right):
                    lo = CHQ[gi][0]
                    q_lo = (a - (S - 1) + P * t) - j0
                    if right:
                        nc.scalar.activation(
                            out=Q[:, q_lo : q_lo + (b - a)],
                            in_=Ws[(h, gi)][:, a - lo : b - lo],
                            func=Copy,
                        )
                    else:
                        nc.vector.tensor_copy(
                            out=Q[:, q_lo : q_lo + (b - a)],
                            in_=Ws[(h, gi)][:, a - lo : b - lo],
                        )
                    if t == 0:
                        # row 0 = alpha*j (K row 0 holds j = c-2047); run it
                        # on the same engine as the copy so the overwrite of
                        # row 0 is ordered by the queue.
                        if right:
                            nc.scalar.activation(
                                out=Q[0:1, q_lo : q_lo + (b - a)],
                                in_=Ks[gi][0:1, a - lo : b - lo],
                                func=Copy,
                                scale=A2[0:1, h : h + 1],
                            )
                        else:
                            nc.vector.tensor_scalar_mul(
                                Q[0:1, q_lo : q_lo + (b - a)],
                                Ks[gi][0:1, a - lo : b - lo],
                                A2[0:1, h : h + 1],
                            )
                if not right:  # col 0 = alpha * i
                    nc.vector.tensor_copy(out=Q[:, 0:1], in_=Rs[h][:, t : t + 1])
                ring.dma_start(
                    out=out_d[h, P * t : P * (t + 1), j0 : j0 + HW], in_=Q[:]
                )

            # --- schedule ---
            compute_chunk(0)  # c2
            for h in range(H_LOC):
                emit_half(h, 0, right=False)  # left t=0 needs only c2
            compute_chunk(1)  # c1
            compute_chunk(2)  # c3
            compute_chunk(3)  # c0
            # rights t=8..15 need {c1, c2}; rights t=0..7 need {c2, c3};
            # lefts t=1..8 need {c1, c2}; lefts t=9..15 need {c0, c1}.
            for t in list(range(8, NT)) + list(range(0, 8)):
                for h in range(H_LOC):
                    emit_half(h, t, right=True)
            for t in range(1, NT):
                for h in range(H_LOC):
                    emit_half(h, t, right=False)

    nc.compile()
    return nc


def _run(alpha, beta, gamma, **spmd_kwargs):
    """Compile (cached) and run on the 8 NeuronCores; returns BassKernelResults."""
    global _NC
    if _NC is None:
        _NC = _build()
    from concourse import bass_utils

    alpha = np.ascontiguousarray(alpha, dtype=np.float32)
    beta = np.ascontiguousarray(beta, dtype=np.float32)
    gamma = np.ascontiguousarray(gamma, dtype=np.float32)
    in_maps = [
        {
            "alpha": alpha[c * H_LOC : (c + 1) * H_LOC],
            "beta": beta[c * H_LOC : (c + 1) * H_LOC],
            "gamma": gamma[c * H_LOC : (c + 1) * H_LOC],
        }
        for c in range(N_CORES)
    ]
    return bass_utils.run_bass_kernel_spmd(
        _NC, in_maps, core_ids=list(range(N_CORES)), **spmd_kwargs
    )


def kernel(alpha, beta, gamma, seq_len):
    assert int(seq_len) == S, f"kernel hardcodes seq_len={S}, got {seq_len}"
    res = _run(alpha, beta, gamma)
    return np.concatenate([r["out"] for r in res.results], axis=0)


# revision 12
# speedup vs baseline: 2.3214x; 1.0960x over previous
"""Bidirectional-ALiBi bias kernel for Trainium2 (Bass/Tile), 8-core SPMD.

Computes out[h, i, j] = |j - i| * m where m = alpha[h] on the first
row/column, gamma[h] above the diagonal, beta[h] below it, and 0 on the
(non-edge) diagonal.  Output [16, 2048, 2048] f32, sharded 2 heads/core.

Strategy: every interior row i is a shifted window of the per-head
profile V(k) = gamma*max(k,0) + beta*max(-k,0), k = j - i.  Each core
computes, per head, THREE overlapping 2047-col diagonalized tiles
W[p, c-lo] = V(c - p - 2047) with lo in {0, 1024, 2048}.  The overlap
is chosen so that for every 128-row block t:
  right half (cols 1024..2047) = ONE contiguous slice of W_B (t>=8)
      or W_C (t<8)  -> DMA'd directly, zero copies;
  left half (cols 0..1023) = one slice of W_A/W_B/W_C plus col 0
      (alpha*i) -> assembled by a single DVE copy + tiny patches into
      a QL tile, then DMA'd.
Block 0 (row 0 = alpha*j) gets both halves assembled with its row-0
patched; all other halves read shared tiles.  Every DMA descriptor in
the kernel is a 4 KB row: HBM write efficiency at 8 KB row stride is
set by descriptor size (4-8 KB descriptors sustain ~400 GB/s; sub-1 KB
descriptors pay a DRAM row-activation per row, <512 B an SDMA
read-modify-write, collapsing throughput to 150-250 GB/s).

Engine placement (respects the DVE/GpSimd shared-SBUF-port lock, and
keeps both HWDGE rings' trigger queues self-paced):
  gpsimd: one master K iota, coefficient partition_broadcasts, IB iota
  ACT (nc.scalar): K chunk derives (K +- 1024), ALL T2 = relu(gamma*K)
      ops (ACT contends with nobody), right-half DMA triggers
  DVE: W = max(-beta*K, T2), left-half assembly, patches
  SP ring (nc.sync): coefficient loads + left-half DMA triggers
Left and right streams are 16.8 MB each - the rings stay balanced.

Hardware notes (from NTFF profiling): 16 SDMA engines; HBM-per-core
limit ~358-420 GB/s; 33.6 MB/core of output writes set a ~84-94 us
roofline, plus ~6.5 us fixed NEFF start barrier, ~7 us of pipeline
fill (coef DMA latency + first chunk), and ~4 us end-barrier drain.
"""

import numpy as np

H = 16
S = 2048
P = 128
N_CORES = 8
H_LOC = H // N_CORES  # 2 heads per core
NT = S // P  # 16 row blocks per head
HW = 1024  # half-row width
CW = 2047  # chunk width

# chunk lo offsets: A=[0,2047), B=[1024,3071), C=[2048,4095)
LO_A, LO_B, LO_C = 0, 1024, 2048

_NC = None


def _build():
    import concourse.bacc as bacc
    import concourse.mybir as mybir
    from concourse.tile import TileContext

    f32 = mybir.dt.float32
    Copy = mybir.ActivationFunctionType.Copy
    Ident = mybir.ActivationFunctionType.Identity
    Relu = mybir.ActivationFunctionType.Relu
    mult, amax = mybir.AluOpType.mult, mybir.AluOpType.max
    nc = bacc.Bacc("TRN2", target_bir_lowering=False, debug=False)

    alpha_d = nc.dram_tensor("alpha", [H_LOC], f32, kind="ExternalInput").ap()
    beta_d = nc.dram_tensor("beta", [H_LOC], f32, kind="ExternalInput").ap()
    gamma_d = nc.dram_tensor("gamma", [H_LOC], f32, kind="ExternalInput").ap()
    out_d = nc.dram_tensor("out", [H_LOC, S, S], f32, kind="ExternalOutput").ap()

    # left half of block t reads c in [2048-128t, 3071-128t); right half
    # c in [3071-128t, 4095-128t).  Serving chunk (single slice each):
    left_lo = lambda t: LO_C if t == 0 else (LO_B if t < 8 else LO_A)
    right_lo = lambda t: LO_C if t < 8 else LO_B

    with TileContext(nc) as tc:
        with (
            tc.tile_pool(name="coef", bufs=1) as cpool,
            tc.tile_pool(name="kpool", bufs=3) as kpool,
            tc.tile_pool(name="tpool", bufs=1) as tpool,
            tc.tile_pool(name="wpool", bufs=3) as wpool,
            tc.tile_pool(name="qlpool", bufs=5) as qlpool,
            tc.tile_pool(name="qrpool", bufs=1) as qrpool,
        ):
            # --- coefficient loads: 3 single-descriptor DMAs, split across
            # both rings so they complete in parallel; broadcast on-chip.
            CG1 = cpool.tile([1, H_LOC], f32, tag="CG1")
            nc.sync.dma_start(out=CG1[0:1, :], in_=gamma_d.unsqueeze(0))
            CB1 = cpool.tile([1, H_LOC], f32, tag="CB1")
            nc.scalar.dma_start(out=CB1[0:1, :], in_=beta_d.unsqueeze(0))
            CA1 = cpool.tile([1, H_LOC], f32, tag="CA1")
            nc.sync.dma_start(out=CA1[0:1, :], in_=alpha_d.unsqueeze(0))

            # --- master K iota first on gpsimd (no input deps) ---
            # K_B[p, x] = (LO_B + x) - p - 2047
            KB = kpool.tile([P, CW], f32, tag="K")
            nc.gpsimd.iota(
                KB[:],
                pattern=[[1, CW]],
                base=LO_B - (S - 1),
                channel_multiplier=-1,
                allow_small_or_imprecise_dtypes=True,
            )
            G2 = cpool.tile([P, H_LOC], f32, tag="G2")
            nc.gpsimd.partition_broadcast(G2[:], CG1[0:1, :])
            B2 = cpool.tile([P, H_LOC], f32, tag="B2")
            nc.gpsimd.partition_broadcast(B2[:], CB1[0:1, :])
            A2 = cpool.tile([P, H_LOC], f32, tag="A2")
            nc.gpsimd.partition_broadcast(A2[:], CA1[0:1, :])
            IB = cpool.tile([P, NT], f32, tag="IB")
            nc.gpsimd.iota(
                IB[:],
                pattern=[[P, NT]],
                base=0,
                channel_multiplier=1,
                allow_small_or_imprecise_dtypes=True,
            )
            bias_p = cpool.tile([P, 1], f32, tag="bias_p")
            nc.gpsimd.memset(bias_p[:], float(HW))
            bias_n = cpool.tile([P, 1], f32, tag="bias_n")
            nc.gpsimd.memset(bias_n[:], float(-HW))

            NB2 = cpool.tile([P, H_LOC], f32, tag="NB2")
            nc.vector.tensor_scalar_mul(NB2[:], B2[:], -1.0)
            Rs = {}
            for h in range(H_LOC):
                Rh = cpool.tile([P, NT], f32, tag=f"Rs{h}")
                nc.vector.tensor_scalar_mul(Rh[:], IB[:], A2[:, h : h + 1])
                Rs[h] = Rh

            Ks = {LO_B: KB}
            Ws = {}

            def derive_k(lo, bias):  # ACT: K_lo = K_B + (lo - LO_B)
                Kg = kpool.tile([P, CW], f32, tag="K")
                nc.scalar.activation(out=Kg[:], in_=KB[:], func=Ident, bias=bias[:])
                Ks[lo] = Kg

            def t2(lo, h):  # ACT: T2 = relu(gamma * K)
                T2t = tpool.tile([P, CW], f32, tag=f"T2{h}")
                nc.scalar.activation(
                    out=T2t[:], in_=Ks[lo][:], func=Relu, scale=G2[:, h : h + 1]
                )
                return T2t

            def wop(lo, h, T2t):  # DVE: W = max(-beta*K, T2) == V(k)
                Wt = wpool.tile([P, CW], f32, tag=f"W{h}")
                nc.vector.scalar_tensor_tensor(
                    out=Wt[:],
                    in0=Ks[lo][:],
                    scalar=NB2[:, h : h + 1],
                    in1=T2t[:],
                    op0=mult,
                    op1=amax,
                )
                Ws[(h, lo)] = Wt

            def emit_right_direct(h, t):
                lo = right_lo(t)
                a = 3071 - 128 * t  # c of j=1024
                nc.scalar.dma_start(
                    out=out_d[h, P * t : P * (t + 1), HW:S],
                    in_=Ws[(h, lo)][:, a - lo : a - lo + HW],
                )

            def emit_right0(h):
                # block 0 right half: row 0 must read alpha*j -> assemble.
                # All ops on ACT so the row-0 overwrite is queue-ordered.
                lo = LO_C
                QR = qrpool.tile([P, HW], f32, tag=f"QR{h}")
                nc.scalar.activation(
                    out=QR[:], in_=Ws[(h, lo)][:, 3071 - lo : 3071 - lo + HW], func=Copy
                )
                nc.scalar.activation(
                    out=QR[0:1, :],
                    in_=Ks[lo][0:1, 3071 - lo : 3071 - lo + HW],
                    func=Copy,
                    scale=A2[0:1, h : h + 1],
                )
                nc.scalar.dma_start(out=out_d[h, 0:P, HW:S], in_=QR[:])

            def emit_left(h, t):
                # col 0 = alpha*i, cols 1..1023 = W slice; t=0 row 0 = alpha*j
                lo = left_lo(t)
                a = 2048 - 128 * t  # c of j=1
                QL = qlpool.tile([P, HW], f32, tag=f"QL{h}")
                nc.vector.tensor_copy(
                    out=QL[:, 1:HW], in_=Ws[(h, lo)][:, a - lo : a - lo + HW - 1]
                )
                if t == 0:
                    nc.vector.tensor_scalar_mul(
                        QL[0:1, 1:HW],
                        Ks[lo][0:1, a - lo : a - lo + HW - 1],
                        A2[0:1, h : h + 1],
                    )
                nc.vector.tensor_copy(out=QL[:, 0:1], in_=Rs[h][:, t : t + 1])
                nc.sync.dma_start(out=out_d[h, P * t : P * (t + 1), 0:HW], in_=QL[:])

            # --- schedule (code order == per-engine queue order) ---
            T2b0 = t2(LO_B, 0)
            wop(LO_B, 0, T2b0)
            T2b1 = t2(LO_B, 1)
            wop(LO_B, 1, T2b1)
            for t in range(8, NT):  # rights t=8..15 h0: direct from W_B
                emit_right_direct(0, t)
            derive_k(LO_C, bias_p)
            T2c0 = t2(LO_C, 0)
            wop(LO_C, 0, T2c0)
            for t in range(8, NT):  # rights h1
                emit_right_direct(1, t)
            T2c1 = t2(LO_C, 1)
            for t in range(1, 8):  # lefts t=1..7 h0 (from W_B)
                emit_left(0, t)
            wop(LO_C, 1, T2c1)
            for t in range(1, 8):  # rights t=1..7 h0: direct from W_C
                emit_right_direct(0, t)
            emit_right0(0)
            derive_k(LO_A, bias_n)
            T2a0 = t2(LO_A, 0)
            emit_left(0, 0)
            emit_left(1, 0)
            for t in range(1, 8):  # lefts t=1..7 h1
                emit_left(1, t)
            wop(LO_A, 0, T2a0)
            for t in range(1, 8):  # rights h1
                emit_right_direct(1, t)
            emit_right0(1)
            T2a1 = t2(LO_A, 1)
            wop(LO_A, 1, T2a1)
            for t in range(8, NT):  # lefts t=8..15 (from W_A)
                emit_left(0, t)
            for t in range(8, NT):
                emit_left(1, t)

    nc.compile()
    return nc


def _run(alpha, beta, gamma, **spmd_kwargs):
    """Compile (cached) and run on the 8 NeuronCores; returns BassKernelResults."""
    global _NC
    if _NC is None:
        _NC = _build()
    from concourse import bass_utils

    alpha = np.ascontiguousarray(alpha, dtype=np.float32)
    beta = np.ascontiguousarray(beta, dtype=np.float32)
    gamma = np.ascontiguousarray(gamma, dtype=np.float32)
    in_maps = [
        {
            "alpha": alpha[c * H_LOC : (c + 1) * H_LOC],
            "beta": beta[c * H_LOC : (c + 1) * H_LOC],
            "gamma": gamma[c * H_LOC : (c + 1) * H_LOC],
        }
        for c in range(N_CORES)
    ]
    return bass_utils.run_bass_kernel_spmd(
        _NC, in_maps, core_ids=list(range(N_CORES)), **spmd_kwargs
    )


def kernel(alpha, beta, gamma, seq_len):
    assert int(seq_len) == S, f"kernel hardcodes seq_len={S}, got {seq_len}"
    res = _run(alpha, beta, gamma)
    return np.concatenate([r["out"] for r in res.results], axis=0)


# revision 13
# speedup vs baseline: 2.4708x; 1.0643x over previous
"""Bidirectional-ALiBi bias kernel for Trainium2 (Bass/Tile), 8-core SPMD.

Computes out[h, i, j] = |j - i| * m where m = alpha[h] on the first
row/column, gamma[h] above the diagonal, beta[h] below it, and 0 on the
(non-edge) diagonal.  Output [16, 2048, 2048] f32, sharded 2 heads/core.

Strategy: every interior row i is a shifted window of the per-head
profile V(k) = gamma*max(k,0) + beta*max(-k,0), k = j - i.  Each core
computes, per head, THREE overlapping 2047-col diagonalized tiles
W[p, c-lo] = V(c - p - 2047) with lo in {0, 1024, 2048}.  The overlap
is chosen so that for every 128-row block t:
  right half (cols 1024..2047) = ONE contiguous slice of W_B (t>=8)
      or W_C (t<8)  -> DMA'd directly, zero copies;
  left half (cols 0..1023) = one slice of W_A/W_B/W_C plus col 0
      (alpha*i) -> assembled by a single DVE copy + tiny patches into
      a QL tile, then DMA'd.
Block 0 (row 0 = alpha*j) gets both halves assembled with its row-0
patched; all other halves read shared tiles.  Every DMA descriptor in
the kernel is a 4 KB row: HBM write efficiency at 8 KB row stride is
set by descriptor size (4-8 KB descriptors sustain ~400 GB/s; sub-1 KB
descriptors pay a DRAM row-activation per row, <512 B an SDMA
read-modify-write, collapsing throughput to 150-250 GB/s).

Engine placement (respects the DVE/GpSimd shared-SBUF-port lock, and
keeps both HWDGE rings' trigger queues self-paced):
  gpsimd: one master K iota, coefficient partition_broadcasts, IB iota
  ACT (nc.scalar): K chunk derives (K +- 1024), ALL T2 = relu(gamma*K)
      ops (ACT contends with nobody), right-half DMA triggers
  DVE: W = max(-beta*K, T2), left-half assembly, patches
  SP ring (nc.sync): coefficient loads + left-half DMA triggers
Left and right streams are 16.8 MB each - the rings stay balanced.

Hardware notes (from NTFF profiling): 16 SDMA engines; HBM-per-core
limit ~358-420 GB/s; 33.6 MB/core of output writes set a ~84-94 us
roofline, plus ~6.5 us fixed NEFF start barrier, ~7 us of pipeline
fill (coef DMA latency + first chunk), and ~4 us end-barrier drain.
"""

import numpy as np

H = 16
S = 2048
P = 128
N_CORES = 8
H_LOC = H // N_CORES  # 2 heads per core
NT = S // P  # 16 row blocks per head
HW = 1024  # half-row width
CW = 2047  # chunk width

# chunk lo offsets: A=[0,2047), B=[1024,3071), C=[2048,4095)
LO_A, LO_B, LO_C = 0, 1024, 2048

_NC = None


def _build():
    import concourse.bacc as bacc
    import concourse.mybir as mybir
    from concourse.tile import TileContext

    f32 = mybir.dt.float32
    Copy = mybir.ActivationFunctionType.Copy
    Ident = mybir.ActivationFunctionType.Identity
    Relu = mybir.ActivationFunctionType.Relu
    mult, amax = mybir.AluOpType.mult, mybir.AluOpType.max
    nc = bacc.Bacc("TRN2", target_bir_lowering=False, debug=False)

    alpha_d = nc.dram_tensor("alpha", [H_LOC], f32, kind="ExternalInput").ap()
    beta_d = nc.dram_tensor("beta", [H_LOC], f32, kind="ExternalInput").ap()
    gamma_d = nc.dram_tensor("gamma", [H_LOC], f32, kind="ExternalInput").ap()
    out_d = nc.dram_tensor("out", [H_LOC, S, S], f32, kind="ExternalOutput").ap()

    # left half of block t reads c in [2048-128t, 3071-128t); right half
    # c in [3071-128t, 4095-128t).  Serving chunk (single slice each):
    left_lo = lambda t: LO_C if t == 0 else (LO_B if t < 8 else LO_A)
    right_lo = lambda t: LO_C if t < 8 else LO_B

    with TileContext(nc) as tc:
        with (
            tc.tile_pool(name="coef", bufs=1) as cpool,
            tc.tile_pool(name="kpool", bufs=3) as kpool,
            tc.tile_pool(name="tpool", bufs=1) as tpool,
            tc.tile_pool(name="wpool", bufs=3) as wpool,
            tc.tile_pool(name="qlpool", bufs=5) as qlpool,
            tc.tile_pool(name="qrpool", bufs=1) as qrpool,
        ):
            # --- coefficient loads: partition-broadcast DMAs.  These put a
            # descriptor on every SDMA engine, which makes the completion
            # semaphore fire ~10.3us in; a minimal 2-descriptor DMA leaves
            # 14 engines idle and its semaphore was measured to arrive at
            # ~18.5us, stalling the whole compute chain.
            G2 = cpool.tile([P, H_LOC], f32, tag="G2")
            nc.sync.dma_start(out=G2[:], in_=gamma_d.partition_broadcast(P))
            B2 = cpool.tile([P, H_LOC], f32, tag="B2")
            nc.scalar.dma_start(out=B2[:], in_=beta_d.partition_broadcast(P))
            A2 = cpool.tile([P, H_LOC], f32, tag="A2")
            nc.sync.dma_start(out=A2[:], in_=alpha_d.partition_broadcast(P))

            # --- master K iota first on gpsimd (no input deps) ---
            # K_B[p, x] = (LO_B + x) - p - 2047
            KB = kpool.tile([P, CW], f32, tag="K")
            nc.gpsimd.iota(
                KB[:],
                pattern=[[1, CW]],
                base=LO_B - (S - 1),
                channel_multiplier=-1,
                allow_small_or_imprecise_dtypes=True,
            )
            IB = cpool.tile([P, NT], f32, tag="IB")
            nc.gpsimd.iota(
                IB[:],
                pattern=[[P, NT]],
                base=0,
                channel_multiplier=1,
                allow_small_or_imprecise_dtypes=True,
            )
            bias_p = cpool.tile([P, 1], f32, tag="bias_p")
            nc.gpsimd.memset(bias_p[:], float(HW))
            bias_n = cpool.tile([P, 1], f32, tag="bias_n")
            nc.gpsimd.memset(bias_n[:], float(-HW))

            NB2 = cpool.tile([P, H_LOC], f32, tag="NB2")
            nc.vector.tensor_scalar_mul(NB2[:], B2[:], -1.0)
            Rs = {}
            for h in range(H_LOC):
                Rh = cpool.tile([P, NT], f32, tag=f"Rs{h}")
                nc.vector.tensor_scalar_mul(Rh[:], IB[:], A2[:, h : h + 1])
                Rs[h] = Rh

            Ks = {LO_B: KB}
            Ws = {}

            def derive_k(lo, bias):  # ACT: K_lo = K_B + (lo - LO_B)
                Kg = kpool.tile([P, CW], f32, tag="K")
                nc.scalar.activation(out=Kg[:], in_=KB[:], func=Ident, bias=bias[:])
                Ks[lo] = Kg

            def t2(lo, h):  # ACT: T2 = relu(gamma * K)
                T2t = tpool.tile([P, CW], f32, tag=f"T2{h}")
                nc.scalar.activation(
                    out=T2t[:], in_=Ks[lo][:], func=Relu, scale=G2[:, h : h + 1]
                )
                return T2t

            def wop(lo, h, T2t):  # DVE: W = max(-beta*K, T2) == V(k)
                Wt = wpool.tile([P, CW], f32, tag=f"W{h}")
                nc.vector.scalar_tensor_tensor(
                    out=Wt[:],
                    in0=Ks[lo][:],
                    scalar=NB2[:, h : h + 1],
                    in1=T2t[:],
                    op0=mult,
                    op1=amax,
                )
                Ws[(h, lo)] = Wt

            def emit_right_direct(h, t):
                lo = right_lo(t)
                a = 3071 - 128 * t  # c of j=1024
                nc.scalar.dma_start(
                    out=out_d[h, P * t : P * (t + 1), HW:S],
                    in_=Ws[(h, lo)][:, a - lo : a - lo + HW],
                )

            def emit_right0(h):
                # block 0 right half: row 0 must read alpha*j -> assemble.
                # All ops on ACT so the row-0 overwrite is queue-ordered.
                lo = LO_C
                QR = qrpool.tile([P, HW], f32, tag=f"QR{h}")
                nc.scalar.activation(
                    out=QR[:], in_=Ws[(h, lo)][:, 3071 - lo : 3071 - lo + HW], func=Copy
                )
                nc.scalar.activation(
                    out=QR[0:1, :],
                    in_=Ks[lo][0:1, 3071 - lo : 3071 - lo + HW],
                    func=Copy,
                    scale=A2[0:1, h : h + 1],
                )
                nc.scalar.dma_start(out=out_d[h, 0:P, HW:S], in_=QR[:])

            def emit_left(h, t):
                # col 0 = alpha*i, cols 1..1023 = W slice; t=0 row 0 = alpha*j
                lo = left_lo(t)
                a = 2048 - 128 * t  # c of j=1
                QL = qlpool.tile([P, HW], f32, tag=f"QL{h}")
                nc.vector.tensor_copy(
                    out=QL[:, 1:HW], in_=Ws[(h, lo)][:, a - lo : a - lo + HW - 1]
                )
                if t == 0:
                    nc.vector.tensor_scalar_mul(
                        QL[0:1, 1:HW],
                        Ks[lo][0:1, a - lo : a - lo + HW - 1],
                        A2[0:1, h : h + 1],
                    )
                nc.vector.tensor_copy(out=QL[:, 0:1], in_=Rs[h][:, t : t + 1])
                nc.sync.dma_start(out=out_d[h, P * t : P * (t + 1), 0:HW], in_=QL[:])

            # --- schedule (code order == per-engine queue order) ---
            T2b0 = t2(LO_B, 0)
            wop(LO_B, 0, T2b0)
            T2b1 = t2(LO_B, 1)
            wop(LO_B, 1, T2b1)
            for t in range(8, NT):  # rights t=8..15 h0: direct from W_B
                emit_right_direct(0, t)
            derive_k(LO_C, bias_p)
            T2c0 = t2(LO_C, 0)
            wop(LO_C, 0, T2c0)
            for t in range(8, NT):  # rights h1
                emit_right_direct(1, t)
            T2c1 = t2(LO_C, 1)
            for t in range(1, 8):  # lefts t=1..7 h0 (from W_B)
                emit_left(0, t)
            wop(LO_C, 1, T2c1)
            for t in range(1, 8):  # rights t=1..7 h0: direct from W_C
                emit_right_direct(0, t)
            emit_right0(0)
            derive_k(LO_A, bias_n)
            T2a0 = t2(LO_A, 0)
            emit_left(0, 0)
            emit_left(1, 0)
            for t in range(1, 8):  # lefts t=1..7 h1
                emit_left(1, t)
            wop(LO_A, 0, T2a0)
            for t in range(1, 8):  # rights h1
                emit_right_direct(1, t)
            emit_right0(1)
            T2a1 = t2(LO_A, 1)
            wop(LO_A, 1, T2a1)
            for t in range(8, NT):  # lefts t=8..15 (from W_A)
                emit_left(0, t)
            for t in range(8, NT):
                emit_left(1, t)

    nc.compile()
    return nc


def _run(alpha, beta, gamma, **spmd_kwargs):
    """Compile (cached) and run on the 8 NeuronCores; returns BassKernelResults."""
    global _NC
    if _NC is None:
        _NC = _build()
    from concourse import bass_utils

    alpha = np.ascontiguousarray(alpha, dtype=np.float32)
    beta = np.ascontiguousarray(beta, dtype=np.float32)
    gamma = np.ascontiguousarray(gamma, dtype=np.float32)
    in_maps = [
        {
            "alpha": alpha[c * H_LOC : (c + 1) * H_LOC],
            "beta": beta[c * H_LOC : (c + 1) * H_LOC],
            "gamma": gamma[c * H_LOC : (c + 1) * H_LOC],
        }
        for c in range(N_CORES)
    ]
    return bass_utils.run_bass_kernel_spmd(
        _NC, in_maps, core_ids=list(range(N_CORES)), **spmd_kwargs
    )


def kernel(alpha, beta, gamma, seq_len):
    assert int(seq_len) == S, f"kernel hardcodes seq_len={S}, got {seq_len}"
    res = _run(alpha, beta, gamma)
    return np.concatenate([r["out"] for r in res.results], axis=0)


# revision 17
# speedup vs baseline: 2.4954x; 1.0100x over previous
"""Bidirectional-ALiBi bias kernel for Trainium2 (Bass/Tile), 8-core SPMD.

Computes out[h, i, j] = |j - i| * m where m = alpha[h] on the first
row/column, gamma[h] above the diagonal, beta[h] below it, and 0 on the
(non-edge) diagonal.  Output [16, 2048, 2048] f32, sharded 2 heads/core.

Strategy: every interior row i is a shifted window of the per-head
profile V(k) = gamma*max(k,0) + beta*max(-k,0), k = j - i.  Each core
computes, per head, THREE overlapping 2047-col diagonalized tiles
W[p, c-lo] = V(c - p - 2047) with lo in {0, 1024, 2048}.  The overlap
is chosen so that for every 128-row block t:
  right half (cols 1024..2047) = ONE contiguous slice of W_B (t>=8)
      or W_C (t<8)  -> DMA'd directly, zero copies;
  left half (cols 0..1023) = one slice of W_A/W_B/W_C plus col 0
      (alpha*i) -> assembled by a single DVE copy + tiny patches into
      a QL tile, then DMA'd.
Block 0 (row 0 = alpha*j) gets both halves assembled with its row-0
patched; all other halves read shared tiles.  Every DMA descriptor in
the kernel is a 4 KB row: HBM write efficiency at 8 KB row stride is
set by descriptor size (4-8 KB descriptors sustain ~400 GB/s; sub-1 KB
descriptors pay a DRAM row-activation per row, <512 B an SDMA
read-modify-write, collapsing throughput to 150-250 GB/s).

Engine placement (respects the DVE/GpSimd shared-SBUF-port lock, and
keeps both HWDGE rings' trigger queues self-paced):
  gpsimd: one master K iota, coefficient partition_broadcasts, IB iota
  ACT (nc.scalar): K chunk derives (K +- 1024), ALL T2 = relu(gamma*K)
      ops (ACT contends with nobody), right-half DMA triggers
  DVE: W = max(-beta*K, T2), left-half assembly, patches
  SP ring (nc.sync): coefficient loads + left-half DMA triggers
Left and right streams are 16.8 MB each - the rings stay balanced.

Hardware notes (from NTFF profiling): 16 SDMA engines; HBM-per-core
limit ~358-420 GB/s; 33.6 MB/core of output writes set a ~84-94 us
roofline, plus ~6.5 us fixed NEFF start barrier, ~7 us of pipeline
fill (coef DMA latency + first chunk), and ~4 us end-barrier drain.
"""

import numpy as np

H = 16
S = 2048
P = 128
N_CORES = 8
H_LOC = H // N_CORES  # 2 heads per core
NT = S // P  # 16 row blocks per head
HW = 1024  # half-row width
CW = 2047  # chunk width

# chunk lo offsets: A=[0,2047), B=[1024,3071), C=[2048,4095)
LO_A, LO_B, LO_C = 0, 1024, 2048

_NC = None


def _build():
    import concourse.bacc as bacc
    import concourse.mybir as mybir
    from concourse.tile import TileContext

    f32 = mybir.dt.float32
    Copy = mybir.ActivationFunctionType.Copy
    Ident = mybir.ActivationFunctionType.Identity
    Relu = mybir.ActivationFunctionType.Relu
    mult, amax = mybir.AluOpType.mult, mybir.AluOpType.max
    nc = bacc.Bacc("TRN2", target_bir_lowering=False, debug=False)

    alpha_d = nc.dram_tensor("alpha", [H_LOC], f32, kind="ExternalInput").ap()
    beta_d = nc.dram_tensor("beta", [H_LOC], f32, kind="ExternalInput").ap()
    gamma_d = nc.dram_tensor("gamma", [H_LOC], f32, kind="ExternalInput").ap()
    out_d = nc.dram_tensor("out", [H_LOC, S, S], f32, kind="ExternalOutput").ap()

    # left half of block t reads c in [2048-128t, 3071-128t); right half
    # c in [3071-128t, 4095-128t).  Serving chunk (single slice each):
    left_lo = lambda t: LO_C if t == 0 else (LO_B if t < 8 else LO_A)
    right_lo = lambda t: LO_C if t < 8 else LO_B

    with TileContext(nc) as tc:
        with (
            tc.tile_pool(name="coef", bufs=1) as cpool,
            tc.tile_pool(name="kpool", bufs=3) as kpool,
            tc.tile_pool(name="tpool", bufs=1) as tpool,
            tc.tile_pool(name="wpool", bufs=3) as wpool,
            tc.tile_pool(name="qlpool", bufs=6) as qlpool,
            tc.tile_pool(name="qrpool", bufs=1) as qrpool,
        ):
            # --- coefficient loads: partition-broadcast DMAs.  These put a
            # descriptor on every SDMA engine, which makes the completion
            # semaphore fire ~10.3us in; a minimal 2-descriptor DMA leaves
            # 14 engines idle and its semaphore was measured to arrive at
            # ~18.5us, stalling the whole compute chain.
            G2 = cpool.tile([P, H_LOC], f32, tag="G2")
            nc.sync.dma_start(out=G2[:], in_=gamma_d.partition_broadcast(P))
            B2 = cpool.tile([P, H_LOC], f32, tag="B2")
            nc.scalar.dma_start(out=B2[:], in_=beta_d.partition_broadcast(P))
            A2 = cpool.tile([P, H_LOC], f32, tag="A2")
            nc.sync.dma_start(out=A2[:], in_=alpha_d.partition_broadcast(P))

            # --- master K iota first on gpsimd (no input deps).  Emitted in
            # two parts (cols [1023,2047) first) so the first T2/W ops can
            # start ~2us earlier than a single [128,2047] iota would allow.
            # K_B[p, x] = (LO_B + x) - p - 2047
            SPL = 1023  # split point; [SPL:CW] is exactly what right t=8 reads
            KB = kpool.tile([P, CW], f32, tag="K")
            nc.gpsimd.iota(
                KB[:, SPL:CW],
                pattern=[[1, CW - SPL]],
                base=LO_B + SPL - (S - 1),
                channel_multiplier=-1,
                allow_small_or_imprecise_dtypes=True,
            )
            nc.gpsimd.iota(
                KB[:, 0:SPL],
                pattern=[[1, SPL]],
                base=LO_B - (S - 1),
                channel_multiplier=-1,
                allow_small_or_imprecise_dtypes=True,
            )
            IB = cpool.tile([P, NT], f32, tag="IB")
            nc.gpsimd.iota(
                IB[:],
                pattern=[[P, NT]],
                base=0,
                channel_multiplier=1,
                allow_small_or_imprecise_dtypes=True,
            )
            bias_p = cpool.tile([P, 1], f32, tag="bias_p")
            nc.gpsimd.memset(bias_p[:], float(HW))
            bias_n = cpool.tile([P, 1], f32, tag="bias_n")
            nc.gpsimd.memset(bias_n[:], float(-HW))

            NB2 = cpool.tile([P, H_LOC], f32, tag="NB2")
            nc.vector.tensor_scalar_mul(NB2[:], B2[:], -1.0)
            Rs = {}
            for h in range(H_LOC):
                Rh = cpool.tile([P, NT], f32, tag=f"Rs{h}")
                nc.vector.tensor_scalar_mul(Rh[:], IB[:], A2[:, h : h + 1])
                Rs[h] = Rh

            Ks = {LO_B: KB}
            Ws = {}

            def derive_k(lo, bias):  # ACT: K_lo = K_B + (lo - LO_B)
                Kg = kpool.tile([P, CW], f32, tag="K")
                nc.scalar.activation(out=Kg[:], in_=KB[:], func=Ident, bias=bias[:])
                Ks[lo] = Kg

            def t2(lo, h, x0=0, x1=CW, T2t=None):  # ACT: T2 = relu(gamma * K)
                if T2t is None:
                    T2t = tpool.tile([P, CW], f32, tag=f"T2{h}")
                nc.scalar.activation(
                    out=T2t[:, x0:x1],
                    in_=Ks[lo][:, x0:x1],
                    func=Relu,
                    scale=G2[:, h : h + 1],
                )
                return T2t

            def wop(lo, h, T2t, x0=0, x1=CW):  # DVE: W = max(-beta*K, T2) == V(k)
                Wt = Ws.get((h, lo))
                if Wt is None:
                    Wt = wpool.tile([P, CW], f32, tag=f"W{h}")
                    Ws[(h, lo)] = Wt
                nc.vector.scalar_tensor_tensor(
                    out=Wt[:, x0:x1],
                    in0=Ks[lo][:, x0:x1],
                    scalar=NB2[:, h : h + 1],
                    in1=T2t[:, x0:x1],
                    op0=mult,
                    op1=amax,
                )

            def emit_right_direct(h, t):
                lo = right_lo(t)
                a = 3071 - 128 * t  # c of j=1024
                nc.scalar.dma_start(
                    out=out_d[h, P * t : P * (t + 1), HW:S],
                    in_=Ws[(h, lo)][:, a - lo : a - lo + HW],
                )

            def emit_right0(h):
                # block 0 right half: row 0 must read alpha*j -> assemble.
                # All ops on ACT so the row-0 overwrite is queue-ordered.
                lo = LO_C
                QR = qrpool.tile([P, HW], f32, tag=f"QR{h}")
                nc.scalar.activation(
                    out=QR[:], in_=Ws[(h, lo)][:, 3071 - lo : 3071 - lo + HW], func=Copy
                )
                nc.scalar.activation(
                    out=QR[0:1, :],
                    in_=Ks[lo][0:1, 3071 - lo : 3071 - lo + HW],
                    func=Copy,
                    scale=A2[0:1, h : h + 1],
                )
                nc.scalar.dma_start(out=out_d[h, 0:P, HW:S], in_=QR[:])

            def emit_left(h, t):
                # col 0 = alpha*i, cols 1..1023 = W slice; t=0 row 0 = alpha*j
                lo = left_lo(t)
                a = 2048 - 128 * t  # c of j=1
                QL = qlpool.tile([P, HW], f32, tag=f"QL{h}")
                nc.vector.tensor_copy(
                    out=QL[:, 1:HW], in_=Ws[(h, lo)][:, a - lo : a - lo + HW - 1]
                )
                if t == 0:
                    nc.vector.tensor_scalar_mul(
                        QL[0:1, 1:HW],
                        Ks[lo][0:1, a - lo : a - lo + HW - 1],
                        A2[0:1, h : h + 1],
                    )
                nc.vector.tensor_copy(out=QL[:, 0:1], in_=Rs[h][:, t : t + 1])
                nc.sync.dma_start(out=out_d[h, P * t : P * (t + 1), 0:HW], in_=QL[:])

            # --- schedule (code order == per-engine queue order) ---
            # h0/B computed in two column halves so right t=8 (which reads
            # exactly W_B[:, SPL:CW]) can launch ~3us earlier.
            T2b0 = t2(LO_B, 0, SPL, CW)
            wop(LO_B, 0, T2b0, SPL, CW)
            t2(LO_B, 0, 0, SPL, T2t=T2b0)
            emit_right_direct(0, 8)
            wop(LO_B, 0, T2b0, 0, SPL)
            T2b1 = t2(LO_B, 1)
            wop(LO_B, 1, T2b1)
            for t in range(9, NT):  # remaining rights h0 from W_B
                emit_right_direct(0, t)
            derive_k(LO_C, bias_p)
            T2c0 = t2(LO_C, 0)
            wop(LO_C, 0, T2c0)
            for t in range(8, NT):  # rights h1
                emit_right_direct(1, t)
            T2c1 = t2(LO_C, 1)
            for t in range(1, 8):  # lefts t=1..7 h0 (from W_B)
                emit_left(0, t)
            wop(LO_C, 1, T2c1)
            for t in range(1, 8):  # rights t=1..7 h0: direct from W_C
                emit_right_direct(0, t)
            emit_right0(0)
            derive_k(LO_A, bias_n)
            T2a0 = t2(LO_A, 0)
            emit_left(0, 0)
            emit_left(1, 0)
            for t in range(1, 8):  # lefts t=1..7 h1
                emit_left(1, t)
            wop(LO_A, 0, T2a0)
            for t in range(1, 8):  # rights h1
                emit_right_direct(1, t)
            emit_right0(1)
            T2a1 = t2(LO_A, 1)
            wop(LO_A, 1, T2a1)
            for t in range(8, NT):  # lefts t=8..15 (from W_A)
                emit_left(0, t)
            for t in range(8, NT):
                emit_left(1, t)

    nc.compile()
    return nc


def _run(alpha, beta, gamma, **spmd_kwargs):
    """Compile (cached) and run on the 8 NeuronCores; returns BassKernelResults."""
    global _NC
    if _NC is None:
        _NC = _build()
    from concourse import bass_utils

    alpha = np.ascontiguousarray(alpha, dtype=np.float32)
    beta = np.ascontiguousarray(beta, dtype=np.float32)
    gamma = np.ascontiguousarray(gamma, dtype=np.float32)
    in_maps = [
        {
            "alpha": alpha[c * H_LOC : (c + 1) * H_LOC],
            "beta": beta[c * H_LOC : (c + 1) * H_LOC],
            "gamma": gamma[c * H_LOC : (c + 1) * H_LOC],
        }
        for c in range(N_CORES)
    ]
    return bass_utils.run_bass_kernel_spmd(
        _NC, in_maps, core_ids=list(range(N_CORES)), **spmd_kwargs
    )


def kernel(alpha, beta, gamma, seq_len):
    assert int(seq_len) == S, f"kernel hardcodes seq_len={S}, got {seq_len}"
    res = _run(alpha, beta, gamma)
    return np.concatenate([r["out"] for r in res.results], axis=0)
